# revision 22
# baseline (speedup 1.0000x reference)
"""MDHP-LSTM Trainium2 kernel (8-core batch-data-parallel, Bass/Tile).

Contract: kernel(**inputs) takes the FULL unsharded inputs (same keys as
reference.setup_inputs()) and returns the same pytree as reference():
(outputs [S,B,H], (h_T [B,H], c_T [B,H])).

Strategy: data-parallel over the batch axis across 8 NeuronCores
(B=128 -> 16 per core), weights replicated. On-device layout is
"gate-major"/transposed: hidden+gate indices live on the 128 SBUF
partitions, batch on the free axis, so the per-step elementwise chain
runs with small free-dims and the recurrent state h.T feeds the next
step's matmul stationary operand with no per-step transposes.
"""

import sys

for _p in ("/opt/trn_rl_repo", "/root/.axon_site/_ro/trn_rl_repo"):
    if _p not in sys.path:
        sys.path.insert(0, _p)

import numpy as np

# Low-precision dtype for the recurrent / input matmul operands.
# fp16 (10 mantissa bits) measures ~10x more accurate than bf16 here at
# identical TensorE throughput; psum accumulation and all state stay fp32.
BF16_NP = np.float16

import concourse.bacc as bacc
import concourse.bass as bass
import concourse.mybir as mybir
import concourse.tile as tile
from concourse.bass_utils import run_bass_kernel_spmd

# Problem shapes (hardcoded per contract).
S, B, D, H, M = 512, 128, 256, 512, 16
NCORES = 8
BL = B // NCORES          # 16 batch rows per core
G4 = 4 * H                # 2048 gate columns, order [i | f | o | c]
KC = H // 128             # 4 hidden k-chunks
KD = D // 128             # 2 input k-chunks
MC = G4 // 128            # 16 gate m-chunks
TC = 16                   # time steps per x@W pre-GEMM chunk
NCHUNK = S // TC

F32 = mybir.dt.float32
BF16 = mybir.dt.float16
AF = mybir.ActivationFunctionType

_CACHE = {}


def _build(n_steps=S, out_last_only=False, kc_lim=KC, skip_ew=False):
    nchunk = n_steps // TC
    n_out = TC if out_last_only else n_steps
    nc = bacc.Bacc(None, target_bir_lowering=False, debug=False)

    # ---- DRAM parameters (per-core shards; names are the in_map keys) ----
    d_xt = nc.dram_tensor("xt", [D, n_steps * BL], BF16, kind="ExternalInput")
    d_u = nc.dram_tensor("u", [H, G4], BF16, kind="ExternalInput")
    d_w = nc.dram_tensor("w", [D, G4], BF16, kind="ExternalInput")
    d_bt = nc.dram_tensor("bt", [128, MC], F32, kind="ExternalInput")
    d_h0 = nc.dram_tensor("h0t", [128, KC * BL], F32, kind="ExternalInput")
    d_c0 = nc.dram_tensor("c0t", [128, KC * BL], F32, kind="ExternalInput")
    d_al = nc.dram_tensor("alphat", [M * M, BL], F32, kind="ExternalInput")
    d_be = nc.dram_tensor("betat", [M * M, BL], F32, kind="ExternalInput")
    d_th = nc.dram_tensor("thetat", [M, BL], F32, kind="ExternalInput")
    d_ts = nc.dram_tensor("tspanb", [128, BL], F32, kind="ExternalInput")
    d_am = nc.dram_tensor("amdhp", [M * M, H], F32, kind="ExternalInput")
    d_bm = nc.dram_tensor("bmdhp", [M * M, H], F32, kind="ExternalInput")
    d_cm = nc.dram_tensor("cmdhp", [M, H], F32, kind="ExternalInput")
    d_id = nc.dram_tensor("ident", [128, 128], F32, kind="ExternalInput")

    d_out = nc.dram_tensor("out", [n_out, BL, H], F32, kind="ExternalOutput")
    d_ct = nc.dram_tensor("ct_out", [128, KC * BL], F32, kind="ExternalOutput")

    with tile.TileContext(nc) as tc:
        with (
            tc.tile_pool(name="const", bufs=1) as cpool,
            tc.tile_pool(name="xw", bufs=2) as xwpool,
            tc.tile_pool(name="xt", bufs=2) as xtpool,
            tc.tile_pool(name="work", bufs=3) as wpool,
            tc.tile_pool(name="gacc", bufs=2, space="PSUM") as gpsum,
            tc.tile_pool(name="pre", bufs=2, space="PSUM") as prepsum,
            tc.tile_pool(name="hps", bufs=2, space="PSUM") as hpsum,
            tc.tile_pool(name="mps", bufs=1, space="PSUM") as mpsum,
        ):
            # ---- persistent SBUF ----
            u_sb = cpool.tile([128, KC * G4], BF16, tag="u_sb")
            w_sb = cpool.tile([128, KD * G4], BF16, tag="w_sb")
            bt_sb = cpool.tile([128, MC], F32, tag="bt_sb")
            id_sb = cpool.tile([128, 128], F32, tag="id_sb")
            hT = cpool.tile([128, KC * BL], F32, tag="hT")
            hTb = cpool.tile([128, KC * BL], BF16, tag="hTb")
            cT = cpool.tile([128, KC * BL], F32, tag="cT")
            mdhpT = cpool.tile([128, KC * BL], F32, tag="mdhpT")
            al_sb = cpool.tile([128, 2 * BL], F32, tag="al_sb")
            be_sb = cpool.tile([128, 2 * BL], F32, tag="be_sb")
            th_sb = cpool.tile([M, BL], F32, tag="th_sb")
            ts_sb = cpool.tile([128, BL], F32, tag="ts_sb")
            am_sb = cpool.tile([128, 2 * H], F32, tag="am_sb")
            bm_sb = cpool.tile([128, 2 * H], F32, tag="bm_sb")
            cm_sb = cpool.tile([M, H], F32, tag="cm_sb")

            for kc in range(KC):
                nc.sync.dma_start(u_sb[:, kc * G4:(kc + 1) * G4],
                                  d_u[kc * 128:(kc + 1) * 128, :])
            for kd in range(KD):
                nc.sync.dma_start(w_sb[:, kd * G4:(kd + 1) * G4],
                                  d_w[kd * 128:(kd + 1) * 128, :])
                nc.sync.dma_start(am_sb[:, kd * H:(kd + 1) * H],
                                  d_am[kd * 128:(kd + 1) * 128, :])
                nc.sync.dma_start(bm_sb[:, kd * H:(kd + 1) * H],
                                  d_bm[kd * 128:(kd + 1) * 128, :])
            nc.sync.dma_start(bt_sb[:], d_bt[:])
            nc.sync.dma_start(id_sb[:], d_id[:])
            nc.sync.dma_start(hT[:], d_h0[:])
            nc.sync.dma_start(cT[:], d_c0[:])
            nc.vector.tensor_copy(hTb[:], hT[:])
            for kd in range(KD):
                nc.sync.dma_start(al_sb[:, kd * BL:(kd + 1) * BL],
                                  d_al[kd * 128:(kd + 1) * 128, :])
                nc.sync.dma_start(be_sb[:, kd * BL:(kd + 1) * BL],
                                  d_be[kd * 128:(kd + 1) * 128, :])
            nc.sync.dma_start(th_sb[:], d_th[:])
            nc.sync.dma_start(ts_sb[:], d_ts[:])
            nc.sync.dma_start(cm_sb[:], d_cm[:])

            # ---- mdhp gate (loop-invariant): mdhpT = tanh(A.T@al + C.T@th - ts*(B.T@be)) ----
            for c4 in range(KC):
                p1 = mpsum.tile([128, BL], F32, tag="mp1")
                p2 = mpsum.tile([128, BL], F32, tag="mp2")
                for kd in range(KD):
                    nc.tensor.matmul(
                        p1[:], am_sb[:, kd * H + c4 * 128: kd * H + (c4 + 1) * 128],
                        al_sb[:, kd * BL:(kd + 1) * BL],
                        start=(kd == 0), stop=False)
                nc.tensor.matmul(
                    p1[:], cm_sb[:, c4 * 128:(c4 + 1) * 128], th_sb[:],
                    start=False, stop=True)
                for kd in range(KD):
                    nc.tensor.matmul(
                        p2[:], bm_sb[:, kd * H + c4 * 128: kd * H + (c4 + 1) * 128],
                        be_sb[:, kd * BL:(kd + 1) * BL],
                        start=(kd == 0), stop=(kd == KD - 1))
                tt = wpool.tile([128, BL], F32, tag="m_tt")
                nc.vector.tensor_mul(tt[:], p2[:], ts_sb[:])
                ss = wpool.tile([128, BL], F32, tag="m_ss")
                nc.vector.tensor_sub(ss[:], p1[:], tt[:])
                nc.scalar.activation(mdhpT[:, c4 * BL:(c4 + 1) * BL], ss[:], AF.Tanh)

            # ---- main loop over time chunks ----
            for ch in range(nchunk):
                t0 = ch * TC
                # x.T chunk [kd*128, TC*BL]
                xt_sb = xtpool.tile([128, KD * TC * BL], BF16, tag="xt_sb")
                for kd in range(KD):
                    nc.sync.dma_start(
                        xt_sb[:, kd * TC * BL:(kd + 1) * TC * BL],
                        d_xt[kd * 128:(kd + 1) * 128, t0 * BL:(t0 + TC) * BL])

                # pre-GEMM: xw.T for this chunk, gate-major, bias folded in.
                xw_sb = xwpool.tile([128, TC, MC * BL], F32, tag="xw_sb")
                for mc in range(MC):
                    pg = prepsum.tile([128, TC, BL], F32, tag="pre_ps")
                    for kd in range(KD):
                        nc.tensor.matmul(
                            pg[:],
                            w_sb[:, kd * G4 + mc * 128: kd * G4 + (mc + 1) * 128],
                            xt_sb[:, kd * TC * BL:(kd + 1) * TC * BL],
                            start=(kd == 0), stop=(kd == KD - 1))
                    nc.scalar.add(
                        xw_sb[:, :, mc * BL:(mc + 1) * BL], pg[:],
                        add=bt_sb[:, mc:mc + 1])

                for t in range(TC):
                    # g.T = U.T @ h.T  (64 accumulating matmuls into one psum tile)
                    g_ps = gpsum.tile([128, MC * BL], F32, tag="g_ps")
                    for mc in range(MC):
                        for kc in range(kc_lim):
                            nc.tensor.matmul(
                                g_ps[:, mc * BL:(mc + 1) * BL],
                                u_sb[:, kc * G4 + mc * 128: kc * G4 + (mc + 1) * 128],
                                hTb[:, kc * BL:(kc + 1) * BL],
                                start=(kc == 0), stop=(kc == kc_lim - 1))
                    # g += xw_t
                    g_sb = wpool.tile([128, MC * BL], F32, tag="g_sb")
                    if skip_ew:
                        nc.vector.tensor_add(hT[:], g_ps[:, 0:KC * BL],
                                             xw_sb[:, t, 0:KC * BL])
                        h_ps = hpsum.tile([BL, H], F32, tag="h_ps")
                        for c4 in range(KC):
                            nc.tensor.transpose(
                                h_ps[:, c4 * 128:(c4 + 1) * 128],
                                hT[:, c4 * BL:(c4 + 1) * BL], id_sb[:])
                        h_sb = wpool.tile([BL, H], F32, tag="h_sb")
                        nc.scalar.copy(h_sb[:], h_ps[:])
                        if (not out_last_only) or ch == nchunk - 1:
                            nc.sync.dma_start(
                                d_out[t if out_last_only else t0 + t], h_sb[:])
                        continue
                    nc.vector.tensor_add(g_sb[:], g_ps[:], xw_sb[:, t, :])
                    # activations: free layout [i(0:64) f(64:128) o(128:192) c(192:256)]
                    ifo = wpool.tile([128, 3 * KC * BL], F32, tag="ifo")
                    nc.scalar.activation(ifo[:], g_sb[:, 0:3 * KC * BL], AF.Sigmoid)
                    chat = wpool.tile([128, KC * BL], F32, tag="chat")
                    nc.scalar.activation(chat[:], g_sb[:, 3 * KC * BL:4 * KC * BL],
                                         AF.Tanh)
                    # c' = mdhp * (f*c + i*chat)
                    t1 = wpool.tile([128, KC * BL], F32, tag="t1")
                    nc.vector.tensor_mul(t1[:], ifo[:, KC * BL:2 * KC * BL], cT[:])
                    t2 = wpool.tile([128, KC * BL], F32, tag="t2")
                    nc.vector.tensor_mul(t2[:], ifo[:, 0:KC * BL], chat[:])
                    t3 = wpool.tile([128, KC * BL], F32, tag="t3")
                    nc.vector.tensor_add(t3[:], t1[:], t2[:])
                    nc.vector.tensor_mul(cT[:], t3[:], mdhpT[:])
                    # h' = o * tanh(c')
                    tct = wpool.tile([128, KC * BL], F32, tag="tct")
                    nc.scalar.activation(tct[:], cT[:], AF.Tanh)
                    nc.vector.tensor_mul(hT[:], ifo[:, 2 * KC * BL:3 * KC * BL],
                                         tct[:])
                    nc.vector.tensor_copy(hTb[:], hT[:])
                    # transpose h.T -> h [BL, H] and stream to DRAM
                    h_ps = hpsum.tile([BL, H], F32, tag="h_ps")
                    for c4 in range(KC):
                        nc.tensor.transpose(
                            h_ps[:, c4 * 128:(c4 + 1) * 128],
                            hT[:, c4 * BL:(c4 + 1) * BL], id_sb[:])
                    h_sb = wpool.tile([BL, H], F32, tag="h_sb")
                    nc.scalar.copy(h_sb[:], h_ps[:])
                    if out_last_only:
                        if ch == nchunk - 1:
                            nc.sync.dma_start(d_out[t], h_sb[:])
                    else:
                        nc.sync.dma_start(d_out[t0 + t], h_sb[:])

            nc.sync.dma_start(d_ct[:], cT[:])

    nc.compile()
    return nc


def _prep_in_maps(x, h0, c0, alpha, beta, theta, tspan,
                  A_mdhp, B_mdhp, C_mdhp, W, U, b, n_steps=S):
    """Host-side shard + layout prep (numpy only)."""
    f = np.float32
    ident = np.eye(128, dtype=f)
    # bias gate-major: bt[p, mc] = b[mc*128 + p]
    bt = np.ascontiguousarray(b.reshape(MC, 128).T.astype(f))
    in_maps = []
    for c in range(NCORES):
        bs = slice(c * BL, (c + 1) * BL)

        def gmaj(v):  # [BL, H] -> [128, KC*BL] gate-major state layout
            return np.ascontiguousarray(
                v.T.reshape(KC, 128, BL).transpose(1, 0, 2).reshape(128, KC * BL)
            ).astype(f)

        xt = np.ascontiguousarray(
            x[:n_steps, bs, :].transpose(2, 0, 1).reshape(D, n_steps * BL)
        ).astype(BF16_NP)
        in_maps.append({
            "xt": xt,
            "u": np.ascontiguousarray(U.astype(BF16_NP)),
            "w": np.ascontiguousarray(W.astype(BF16_NP)),
            "bt": bt,
            "h0t": gmaj(h0[bs]),
            "c0t": gmaj(c0[bs]),
            "alphat": np.ascontiguousarray(alpha[bs].T.astype(f)),
            "betat": np.ascontiguousarray(beta[bs].T.astype(f)),
            "thetat": np.ascontiguousarray(theta[bs].T.astype(f)),
            "tspanb": np.ascontiguousarray(
                np.broadcast_to(tspan[bs].astype(f), (128, BL))),
            "amdhp": np.ascontiguousarray(A_mdhp.astype(f)),
            "bmdhp": np.ascontiguousarray(B_mdhp.astype(f)),
            "cmdhp": np.ascontiguousarray(C_mdhp.astype(f)),
            "ident": ident,
        })
    return in_maps


def _concat_weights(W_i, U_i, b_i, W_f, U_f, b_f, W_c, U_c, b_c, W_o, U_o, b_o):
    # gate order on device: [i | f | o | c]
    W = np.concatenate([W_i, W_f, W_o, W_c], axis=1)
    U = np.concatenate([U_i, U_f, U_o, U_c], axis=1)
    b = np.concatenate([b_i, b_f, b_o, b_c], axis=0)
    return W, U, b


def _run(inputs, n_steps=S, trace=False, trace_kwargs=None):
    key = n_steps
    if key not in _CACHE:
        _CACHE[key] = _build(n_steps)
    nc = _CACHE[key]
    W, U, b = _concat_weights(
        inputs["W_i"], inputs["U_i"], inputs["b_i"],
        inputs["W_f"], inputs["U_f"], inputs["b_f"],
        inputs["W_c"], inputs["U_c"], inputs["b_c"],
        inputs["W_o"], inputs["U_o"], inputs["b_o"])
    in_maps = _prep_in_maps(
        np.asarray(inputs["x"]), np.asarray(inputs["h0"]),
        np.asarray(inputs["c0"]), np.asarray(inputs["alpha"]),
        np.asarray(inputs["beta"]), np.asarray(inputs["theta"]),
        np.asarray(inputs["tspan"]), np.asarray(inputs["A_mdhp"]),
        np.asarray(inputs["B_mdhp"]), np.asarray(inputs["C_mdhp"]),
        W, U, b, n_steps=n_steps)
    kwargs = {}
    if trace:
        kwargs = dict(trace=True, trace_kwargs=trace_kwargs or {})
    res = run_bass_kernel_spmd(nc, in_maps, core_ids=list(range(NCORES)), **kwargs)

    outputs = np.empty((n_steps, B, H), np.float32)
    c_T = np.empty((B, H), np.float32)
    for c in range(NCORES):
        bs = slice(c * BL, (c + 1) * BL)
        outputs[:, bs, :] = res.results[c]["out"]
        ct_gm = res.results[c]["ct_out"]  # [128, KC*BL] gate-major
        c_T[bs] = ct_gm.reshape(128, KC, BL).transpose(2, 1, 0).reshape(BL, H)
    h_T = outputs[-1].copy()
    return (outputs, (h_T, c_T)), res


def kernel(**inputs):
    (outputs, (h_T, c_T)), _ = _run(inputs)
    return outputs, (h_T, c_T)
